# revision 17
# baseline (speedup 1.0000x reference)
"""GCNII (8 layers, N=50000, E=800000) on 8 trn2 NeuronCores — v2.

Sharding: nodes partitioned into 8 contiguous ranges (6250/core); edges
partitioned by destination so each core owns the scatter-add for its node
range. Per layer: segmented AllGather of dinv-scaled h (bf16, ping-pong
buffers, overlapped with epilogue compute) -> HBM h_full; each core
dma_gathers the 256B source rows for its edges, scatters them into PSUM
via exact 0/1 one-hot matmuls (bf16, one-hot tiles built in one batched
DVE op per chunk), then applies the dense epilogue with the layer matrix
folded as M = (1-beta)I + beta*W. GCN norm = dinv[src] folded into the
gathered h, dinv[dst] applied as a per-partition scale in the epilogue.
"""
import hashlib
import numpy as np
import ml_dtypes
import concourse.bass as bass
import concourse.mybir as mybir
from concourse import bacc, tile
from concourse.bass_utils import run_bass_kernel_spmd

mdt = mybir.dt
bf16 = ml_dtypes.bfloat16

N = 50000
E = 800000
FIN = 128
HID = 64
L = 8
ALPHA = 0.1
THETA = 0.5
NCORES = 8
NS = N // NCORES            # 6250 nodes per core
NW = (NS + 127) // 128      # 49 windows per core
NSPAD = NW * 128            # 6272
CW = 7                      # windows per chunk
CHUNKS = NW // CW           # 7 chunks
HALF = 32768                # int16 gather index split
# chunk -> local row range
CHROW = [min(c * CW * 128, NS) for c in range(CHUNKS + 1)]
# segments (for the pipelined allgather), as chunk ranges
SEG_CH = [(0, 2), (2, 4), (4, 6), (6, 7)]
NSEG = len(SEG_CH)
SEGROW = [(CHROW[a], CHROW[b]) for a, b in SEG_CH]


def _preprocess(x, edge_index, w_in, b_in, conv_w, w_out, b_out):
    row = np.asarray(edge_index[0], dtype=np.int64)
    col = np.asarray(edge_index[1], dtype=np.int64)
    loops = np.arange(N, dtype=np.int64)
    row = np.concatenate([row, loops])
    col = np.concatenate([col, loops])
    deg = np.bincount(col, minlength=N).astype(np.float32)
    dinv = (1.0 / np.sqrt(deg)).astype(np.float32)

    # ---- seg-major renumbering: newpos[c*NS+r] = 8*a_s + c*len_s + (r-a_s)
    starts = np.array([s for s, _ in SEGROW], dtype=np.int64)
    ends = np.array([e for _, e in SEGROW], dtype=np.int64)
    r = np.arange(NS, dtype=np.int64)
    sid = np.searchsorted(ends, r, side="right")
    a, ln = starts[sid], (ends - starts)[sid]
    newpos = (8 * a + (r - a))[None, :] + np.arange(NCORES)[:, None] * ln[None, :]
    newpos = newpos.reshape(-1)       # index by old global id
    nsrc_all = newpos[row].astype(np.int32)

    # ---- per-core edge lists, grouped (window, src-parity)
    # h is stored tight bf16 [N,64] viewed as [N/2,128]; a 256B gather row
    # holds the node PAIR (2k, 2k+1); tiles are parity-pure so the matmul
    # rhs slice (cols 0:64 / 64:128) picks the right node.
    cores = []
    counts = np.zeros((NCORES, NW, 2), dtype=np.int64)
    for c in range(NCORES):
        m = (col >= c * NS) & (col < (c + 1) * NS)
        d = (col[m] - c * NS).astype(np.int32)
        s = nsrc_all[m]
        hi = (s & 1).astype(np.int32)
        win = d >> 7
        key = win * 2 + hi
        o = np.argsort(key, kind="stable")
        d, s, hi, key = d[o], s[o], hi[o], key[o]
        counts[c] = np.bincount(key, minlength=NW * 2).reshape(NW, 2)
        cores.append((d, s, hi, key))

    TLs = -(-counts[:, :, 0].max(axis=0) // 128)
    THs = -(-counts[:, :, 1].max(axis=0) // 128)
    assert (TLs + THs).min() >= 1

    # global tile order: per chunk, all lo tiles (window-major) then all hi
    gidx_lo = np.zeros(NW, dtype=np.int64)
    gidx_hi = np.zeros(NW, dtype=np.int64)
    g = 0
    for sch in range(CHUNKS):
        for w in range(sch * CW, (sch + 1) * CW):
            gidx_lo[w] = g
            g += TLs[w]
        for w in range(sch * CW, (sch + 1) * CW):
            gidx_hi[w] = g
            g += THs[w]
    T = g
    NLO = [int(TLs[s * CW: (s + 1) * CW].sum()) for s in range(CHUNKS)]
    NHI = [int(THs[s * CW: (s + 1) * CW].sum()) for s in range(CHUNKS)]
    CBASE = [int(gidx_lo[s * CW]) for s in range(CHUNKS)]

    # ---- dense weights (shared across cores)
    w_in = np.asarray(w_in, np.float32)
    conv_w = np.asarray(conv_w, np.float32)
    w_out = np.asarray(w_out, np.float32)
    b_in = np.asarray(b_in, np.float32)
    b_out = np.asarray(b_out, np.float32)
    betas = np.log(THETA / np.arange(1, L + 1, dtype=np.float32) + 1.0)
    convT = np.concatenate(
        [((1.0 - betas[l]) * np.eye(HID, dtype=np.float32)
          + betas[l] * conv_w[l]).T for l in range(L)], axis=1)
    iota = np.tile(np.arange(128, dtype=np.float32), (128, 1))
    ident = np.eye(128, dtype=np.float32)
    consts = {
        "w_inT": np.ascontiguousarray(w_in.T).astype(bf16),       # [128, 64]
        "convT": np.ascontiguousarray(convT).astype(bf16),        # [64, L*64]
        "w_outT": np.ascontiguousarray(w_out.T).astype(bf16),     # [64, 64]
        "b_in_rep": np.tile(b_in[None, :], (128, 1)),             # [128,64] f32
        "b_out_rep": np.tile(b_out[None, :], (128, 1)),           # [128,64] f32
        "iota_bf": iota.astype(bf16),
        "ident_bf": ident.astype(bf16),
    }

    x = np.asarray(x, np.float32)
    tileidx = np.where(np.arange(NW)[:, None] >= 0, 0, 0)  # placeholder
    in_maps = []
    for c in range(NCORES):
        d, s, hi, key = cores[c]
        gstart = np.searchsorted(key, np.arange(NW * 2))
        slot = np.arange(len(d)) - gstart[key]
        win = d >> 7
        gidx = np.where(hi == 1, gidx_hi[win], gidx_lo[win])
        tix = (gidx + (slot >> 7)).astype(np.int64)
        rix = (slot & 127).astype(np.int64)
        dstw8 = np.full((128, T), -1, dtype=np.int8)
        dstw8[rix, tix] = (d & 127).astype(np.int8)
        srcv = np.zeros((128, T), dtype=np.int32)
        srcv[rix, tix] = s >> 1
        # idx16[p, 8g+j] = srcv[16j+p, g]  (16-partition wrap)
        idx16 = (srcv.astype(np.int16).T.reshape(T, 8, 16)
                 .transpose(2, 0, 1).reshape(16, 8 * T))

        lidx = np.minimum(c * NS + np.arange(NSPAD), (c + 1) * NS - 1)
        dl = dinv[lidx].reshape(NW, 128).T                       # [128, NW]
        xT = np.zeros((FIN, NSPAD), np.float32)
        xT[:, :NS] = x[c * NS: (c + 1) * NS].T
        in_maps.append(dict(
            consts,
            xT=np.ascontiguousarray(xT).astype(bf16),
            idx16=np.ascontiguousarray(idx16),
            dstw8=np.ascontiguousarray(dstw8),
            dinvs=np.ascontiguousarray(dl),
            dinv09=np.ascontiguousarray((1.0 - ALPHA) * dl),
        ))

    cfg = dict(TLs=TLs, THs=THs, gidx_lo=gidx_lo, gidx_hi=gidx_hi, T=T,
               NLO=NLO, NHI=NHI, CBASE=CBASE)
    return in_maps, cfg


def _build(cfg, reps=1, sim_single=False, ablate=(), nsplit=2, nseg=NSEG,
           single_packet=False):
    ablate = set(ablate)
    TLs, THs = cfg["TLs"], cfg["THs"]
    gidx_lo, gidx_hi, T = cfg["gidx_lo"], cfg["gidx_hi"], cfg["T"]
    NLO, NHI, CBASE = cfg["NLO"], cfg["NHI"], cfg["CBASE"]
    seg_ch = SEG_CH if nseg == NSEG else [(0, CHUNKS)]
    segrow = [(CHROW[a], CHROW[b]) for a, b in seg_ch]

    nc = bacc.Bacc(None, target_bir_lowering=False, num_devices=NCORES,
                   num_swdge_queues=4)

    xT_in = nc.dram_tensor("xT", [FIN, NSPAD], mdt.bfloat16, kind="ExternalInput")
    idx_in = nc.dram_tensor("idx16", [16, 8 * T], mdt.int16, kind="ExternalInput")
    dstw_in = nc.dram_tensor("dstw8", [128, T], mdt.int8, kind="ExternalInput")
    dinvs_in = nc.dram_tensor("dinvs", [128, NW], mdt.float32, kind="ExternalInput")
    dinv09_in = nc.dram_tensor("dinv09", [128, NW], mdt.float32, kind="ExternalInput")
    w_inT_in = nc.dram_tensor("w_inT", [FIN, HID], mdt.bfloat16, kind="ExternalInput")
    convT_in = nc.dram_tensor("convT", [HID, L * HID], mdt.bfloat16, kind="ExternalInput")
    w_outT_in = nc.dram_tensor("w_outT", [HID, HID], mdt.bfloat16, kind="ExternalInput")
    b_in_in = nc.dram_tensor("b_in_rep", [128, HID], mdt.float32, kind="ExternalInput")
    b_out_in = nc.dram_tensor("b_out_rep", [128, HID], mdt.float32, kind="ExternalInput")
    iota_in = nc.dram_tensor("iota_bf", [128, 128], mdt.bfloat16, kind="ExternalInput")
    ident_in = nc.dram_tensor("ident_bf", [128, 128], mdt.bfloat16, kind="ExternalInput")

    out_t = nc.dram_tensor("out", [NS, HID], mdt.bfloat16, kind="ExternalOutput")

    # one bounce tensor per allgather segment so the tile framework's
    # whole-tensor DRAM dependency tracking lets segment s's collective
    # start as soon as segment s's windows are stored (not the whole layer)
    bnc = [nc.dram_tensor(f"bounce{i}", [r1 - r0, HID], mdt.bfloat16)
           for i, (r0, r1) in enumerate(segrow)]
    hf = [nc.dram_tensor(f"h_full{i}", [N // 2, FIN], mdt.bfloat16,
                         addr_space="Shared")
          for i in range(2)]
    seg_of_row = []
    for i, (r0, r1) in enumerate(segrow):
        seg_of_row += [i] * (r1 - r0)

    MAXK = max(NLO[s] + NHI[s] for s in range(CHUNKS))

    with tile.TileContext(nc) as tc, \
         tc.tile_pool(name="const", bufs=1) as cpool, \
         tc.tile_pool(name="gath", bufs=2) as gpool, \
         tc.tile_pool(name="oh", bufs=2) as ohpool, \
         tc.tile_pool(name="work", bufs=3) as wpool, \
         tc.tile_pool(name="ps_sc", bufs=2, space="PSUM") as psum_sc, \
         tc.tile_pool(name="ps_tr", bufs=2, space="PSUM") as psum_tr, \
         tc.tile_pool(name="ps_mm", bufs=2, space="PSUM") as psum_mm:

        # ---- persistent constants ----
        iota_t = cpool.tile([128, 128], mdt.bfloat16)
        nc.sync.dma_start(iota_t[:], iota_in[:])
        ident_t = cpool.tile([128, 128], mdt.bfloat16)
        nc.sync.dma_start(ident_t[:], ident_in[:])
        w_inT_t = cpool.tile([FIN, HID], mdt.bfloat16)
        nc.sync.dma_start(w_inT_t[:], w_inT_in[:])
        convT_t = cpool.tile([HID, L * HID], mdt.bfloat16)
        nc.sync.dma_start(convT_t[:], convT_in[:])
        w_outT_t = cpool.tile([HID, HID], mdt.bfloat16)
        nc.sync.dma_start(w_outT_t[:], w_outT_in[:])
        b_in_t = cpool.tile([128, HID], mdt.float32)
        nc.sync.dma_start(b_in_t[:], b_in_in[:])
        b_out_t = cpool.tile([128, HID], mdt.float32)
        nc.sync.dma_start(b_out_t[:], b_out_in[:])
        dinvs_t = cpool.tile([128, NW], mdt.float32)
        nc.sync.dma_start(dinvs_t[:], dinvs_in[:])
        dinv09_t = cpool.tile([128, NW], mdt.float32)
        nc.sync.dma_start(dinv09_t[:], dinv09_in[:])
        idx_t = cpool.tile([128, 8 * T], mdt.int16)
        for k in range(8):
            nc.sync.dma_start(idx_t[16 * k: 16 * (k + 1), :], idx_in[:])
        dstw8_t = cpool.tile([128, T], mdt.int8)
        nc.sync.dma_start(dstw8_t[:], dstw_in[:])
        xT_t = cpool.tile([FIN, NSPAD], mdt.bfloat16)
        nc.sync.dma_start(xT_t[:], xT_in[:])

        dstw_t = cpool.tile([128, T], mdt.bfloat16)
        nc.vector.tensor_copy(dstw_t[:], dstw8_t[:])

        h_sb = cpool.tile([128, NW * HID], mdt.bfloat16)
        x0s = cpool.tile([128, NW * HID], mdt.bfloat16)

        def store_h(w):
            nrows = min(NS - w * 128, 128)
            r0 = w * 128
            si = seg_of_row[r0]
            s0 = segrow[si][0]
            nc.sync.dma_start(
                bnc[si][r0 - s0: r0 - s0 + nrows, :],
                h_sb[:nrows, w * HID: (w + 1) * HID],
            )

        qctr = [0]

        for rep_i in range(reps):
            def allgather(seg, dst):
                r0, r1 = segrow[seg]
                if sim_single or "collective" in ablate:
                    nc.sync.dma_start(
                        dst[4 * r0: 4 * r0 + (r1 - r0) // 2, :],
                        bnc[seg][:, :].rearrange("(a b) c -> a (b c)", b=2))
                else:
                    nc.gpsimd.collective_compute(
                        "AllGather", mybir.AluOpType.bypass,
                        replica_groups=[list(range(NCORES))],
                        ins=[bnc[seg][:, :]], outs=[dst[4 * r0: 4 * r1, :]],
                    )

            # ---- h0 = relu(x @ w_in.T + b_in); x0s = ALPHA*h0; h' = dinv*h0
            seg_ptr = 0
            for w in range(NW):
                ps = psum_mm.tile([128, HID], mdt.float32, tag="mm")
                nc.tensor.matmul(ps[:], xT_t[:, w * 128: (w + 1) * 128],
                                 w_inT_t[:], start=True, stop=True)
                u = wpool.tile([128, HID], mdt.float32, tag="u")
                nc.vector.tensor_tensor(u[:], ps[:], b_in_t[:], mybir.AluOpType.add)
                h0w = wpool.tile([128, HID], mdt.bfloat16, tag="h0")
                nc.scalar.activation(h0w[:], u[:], mybir.ActivationFunctionType.Relu)
                nc.vector.tensor_scalar_mul(x0s[:, w * HID: (w + 1) * HID],
                                            h0w[:], ALPHA)
                nc.vector.tensor_scalar(
                    h_sb[:, w * HID: (w + 1) * HID], h0w[:],
                    dinvs_t[:, w: w + 1], None, mybir.AluOpType.mult)
                store_h(w)
                while seg_ptr < len(segrow) and (w + 1) * 128 >= segrow[seg_ptr][1]:
                    allgather(seg_ptr, hf[0])
                    seg_ptr += 1

            # ---- layers ----
            for l in range(L):
                src_hf = hf[l % 2]
                dst_hf = hf[(l + 1) % 2]
                seg_ptr = 0
                for s in range(CHUNKS):
                    base = CBASE[s]
                    K = NLO[s] + NHI[s]
                    gt = gpool.tile([128, MAXK, FIN], mdt.bfloat16, tag="g")

                    def one_gather(src_ap, t0, ntiles):
                        if "gather" in ablate or ntiles == 0:
                            return
                        bnds = [t0 + (ntiles * i) // nsplit
                                for i in range(nsplit + 1)]
                        for i in range(nsplit):
                            a2, b2 = bnds[i], bnds[i + 1]
                            if a2 == b2:
                                continue
                            nidx = (b2 - a2) * 128
                            nc.gpsimd.dma_gather(
                                gt[:, a2 - base: b2 - base, :], src_ap,
                                idx_t[:, 8 * a2: 8 * b2], nidx, nidx, FIN,
                                single_packet=single_packet,
                                queue_num=qctr[0] % 4,
                            )
                            qctr[0] += 1

                    one_gather(src_hf[:, :], base, NLO[s] + NHI[s])

                    if "onehot" not in ablate:
                        oh = ohpool.tile([128, MAXK, 128], mdt.bfloat16, tag="oh")
                        nc.vector.tensor_tensor(
                            oh[:, :K, :],
                            iota_t[:].unsqueeze(1).broadcast_to([128, K, 128]),
                            dstw_t[:, base: base + K].unsqueeze(2)
                                .broadcast_to([128, K, 128]),
                            mybir.AluOpType.is_equal)

                    for wi in range(CW):
                        w = s * CW + wi
                        if w >= NW:
                            break
                        ntiles = int(TLs[w] + THs[w])
                        ps = psum_sc.tile([128, HID], mdt.float32, tag="sc")
                        k = 0
                        for p in range(2):
                            TT = int(TLs[w] if p == 0 else THs[w])
                            g0 = int(gidx_lo[w] if p == 0 else gidx_hi[w])
                            for t in range(TT):
                                gg = g0 + t
                                slot = gg - base
                                if "scatter_mm" not in ablate:
                                    oh_ap = (iota_t[:] if "onehot" in ablate
                                             else oh[:, slot, :])
                                    g_ap = (b_in_t[:].bitcast(mdt.bfloat16)[:, :HID]
                                            if "gather" in ablate
                                            else gt[:, slot, p * HID: (p + 1) * HID])
                                    nc.tensor.matmul(
                                        ps[:], oh_ap, g_ap,
                                        start=(k == 0), stop=(k == ntiles - 1))
                                k += 1
                        ps_ap = b_in_t[:] if "scatter_mm" in ablate else ps[:]
                        # z = 0.9*dinv_d*ps + 0.1*x0 ; h = relu(z @ M_l.T)
                        zw1 = wpool.tile([128, HID], mdt.bfloat16, tag="zw1")
                        nc.vector.tensor_scalar(
                            zw1[:], ps_ap, dinv09_t[:, w: w + 1], None,
                            mybir.AluOpType.mult)
                        zw = wpool.tile([128, HID], mdt.bfloat16, tag="zw")
                        nc.vector.tensor_tensor(
                            zw[:], zw1[:], x0s[:, w * HID: (w + 1) * HID],
                            mybir.AluOpType.add)
                        ztp = psum_tr.tile([HID, 128], mdt.bfloat16, tag="tr")
                        nc.tensor.transpose(ztp[:], zw[:], ident_t[:])
                        zt = wpool.tile([HID, 128], mdt.bfloat16, tag="zt")
                        nc.scalar.copy(zt[:], ztp[:])
                        ps2 = psum_mm.tile([128, HID], mdt.float32, tag="mm")
                        nc.tensor.matmul(ps2[:], zt[:],
                                         convT_t[:, l * HID: (l + 1) * HID],
                                         start=True, stop=True)
                        if l < L - 1:
                            nc.scalar.activation(
                                h_sb[:, w * HID: (w + 1) * HID], ps2[:],
                                mybir.ActivationFunctionType.Relu,
                                scale=dinvs_t[:, w: w + 1])
                            store_h(w)
                        else:
                            nc.scalar.activation(
                                h_sb[:, w * HID: (w + 1) * HID], ps2[:],
                                mybir.ActivationFunctionType.Relu)
                    if l < L - 1:
                        while (seg_ptr < len(seg_ch)
                               and s + 1 >= seg_ch[seg_ptr][1]):
                            allgather(seg_ptr, dst_hf)
                            seg_ptr += 1

            # ---- out = h @ w_out.T + b_out ----
            for w in range(NW):
                htp = psum_tr.tile([HID, 128], mdt.bfloat16, tag="tr")
                nc.tensor.transpose(htp[:], h_sb[:, w * HID: (w + 1) * HID],
                                    ident_t[:])
                ht = wpool.tile([HID, 128], mdt.bfloat16, tag="zt")
                nc.scalar.copy(ht[:], htp[:])
                ps3 = psum_mm.tile([128, HID], mdt.float32, tag="mm")
                nc.tensor.matmul(ps3[:], ht[:], w_outT_t[:], start=True, stop=True)
                ow = wpool.tile([128, HID], mdt.bfloat16, tag="ow")
                nc.vector.tensor_tensor(ow[:], ps3[:], b_out_t[:],
                                        mybir.AluOpType.add)
                nrows = min(NS - w * 128, 128)
                nc.sync.dma_start(out_t[w * 128: w * 128 + nrows, :],
                                  ow[:nrows, :])

    nc.finalize()
    return nc


_NC_CACHE = {}


def kernel(**inputs) -> np.ndarray:
    in_maps, cfg = _preprocess(
        inputs["x"], inputs["edge_index"], inputs["w_in"], inputs["b_in"],
        inputs["conv_w"], inputs["w_out"], inputs["b_out"],
    )
    key = hashlib.sha1(np.ascontiguousarray(inputs["edge_index"])).hexdigest()
    if key not in _NC_CACHE:
        _NC_CACHE[key] = _build(cfg)
    nc = _NC_CACHE[key]
    res = run_bass_kernel_spmd(nc, in_maps, list(range(NCORES)))
    out = np.concatenate(
        [res.results[c]["out"].astype(np.float32) for c in range(NCORES)], axis=0)
    return out
